# revision 8
# baseline (speedup 1.0000x reference)
"""Trainium2 Bass kernel for nn_MessageLayer (GNN message passing), 8 NeuronCores.

Reference computation:
    edge_mat = (edge_features @ W + b).reshape(E, 64, 16)
    messages = einsum('emh,eh->em', edge_mat, hidden[edge_sources])
    out      = segment_sum(messages, edge_targets, num_segments=10000)

Algebraic restructure (cuts FLOPs 32x): since aggregation is linear,
    out[n, m] = sum_{f,h} W[f, m*16+h] * C[n, f, h],
    C[n, f, h] = sum_{e: tgt(e)=n} ef[e, f] * hidden[src(e), h]
C is built with one tiny PE matmul per target node (lhsT = that node's edge
features [deg, 32], rhs = gathered source hidden [deg, 16]) — segment
boundaries are known when the kernel is built, so they are baked into the
unrolled instruction stream.  Then out = C @ Wr as 32 accumulating matmuls
against a block-diagonal-packed W.

All matmul operands are bf16 (PSUM accumulation stays fp32): the PE streams
bf16 at 1 cycle/row vs fp32's 4, and DMA bytes halve.  Quantization error
~0.3% is far inside the 2e-2 gate.

Sharding: node-ownership (scatter-reduce by target) — each core receives
exactly the edges targeting its nodes, so no collective is needed.  Nodes
are dealt to cores snake-wise in descending-degree order, so the sorted
per-core segment-length profiles match almost exactly across cores and the
SPMD max-padding (K_j = max over cores of the j-th longest segment) is
minimal.

c_all is stored h-major ([128, 16, NB*32]) so the W-stage moving operand
c_all[:, h, :] is contiguous — a strided (h::16) moving was measured at
~2.7 ns/column vs ~0.8 contiguous.

Hardware constraint discovered empirically: matmuls whose stationary tiles
sit on disjoint row-groups execute CONCURRENTLY, and two concurrent matmuls
draining into the same PSUM bank wedge the device (NRT_EXEC_UNIT_
UNRECOVERABLE).  C-bank assignment is therefore keyed by starting row-group:
matmuls sharing a row-group serialize on the PE and may share a bank;
different row-groups always land in different banks.
"""
import numpy as np
import ml_dtypes
from contextlib import ExitStack

BF16 = ml_dtypes.bfloat16

N_NODES = 10000
N_EDGES = 320000
HID = 16
MSG = 64
EFD = 32
NCORES = 8
# progressive input chunks: small first chunk so the PE starts early
CHUNK_FRACS = (0.0, 0.05, 0.14, 0.28, 0.48, 0.72, 1.0)
NCHUNK = len(CHUNK_FRACS) - 1
CPBUFS = 6                       # PSUM tiles for C banks (+2 for the W stage)

_CACHE = {}


def _build_layout(edge_targets):
    """Per-core segment lists + the shared (SPMD-uniform) layout."""
    deg = np.bincount(edge_targets, minlength=N_NODES)
    order = np.argsort(-deg, kind="stable")      # nodes by degree desc
    node_core = np.empty(N_NODES, dtype=np.int64)
    snake = list(range(NCORES)) + list(range(NCORES - 1, -1, -1))
    for i, n in enumerate(order):
        node_core[n] = snake[i % (2 * NCORES)]

    # bucket edge ids by target, in one pass
    order_e = np.argsort(edge_targets, kind="stable")
    tgt_sorted = edge_targets[order_e]
    uniq, starts = np.unique(tgt_sorted, return_index=True)
    bounds = list(starts) + [len(tgt_sorted)]

    # split all segments to <=32 edges: every position then fits a single
    # 32-row PE group, so all four row-group streams run concurrently
    # (positions spanning 2+ groups serialize pairwise). Host re-adds splits.
    segs_per_core = [[] for _ in range(NCORES)]
    for i, n in enumerate(uniq):
        s, e = bounds[i], bounds[i + 1]
        c = node_core[n]
        while e - s > 32:
            segs_per_core[c].append((int(n), order_e[s:s + 32]))
            s += 32
        segs_per_core[c].append((int(n), order_e[s:e]))
    for c in range(NCORES):
        segs_per_core[c].sort(key=lambda t: -len(t[1]))

    nseg = max(len(s) for s in segs_per_core)
    NPOS = ((nseg + 127) // 128) * 128
    K = np.ones(NPOS, dtype=np.int64)
    for segs in segs_per_core:
        for j, (_, e) in enumerate(segs):
            K[j] = max(K[j], len(e))

    # pack positions into 128-row tiles at 32-aligned row bases
    t_j = np.zeros(NPOS, dtype=np.int64)
    r_j = np.zeros(NPOS, dtype=np.int64)
    tile, row = 0, 0
    for j in range(NPOS):
        s = (int(K[j]) + 31) // 32
        if row + 32 * s > 128:
            tile += 1
            row = 0
        t_j[j], r_j[j] = tile, row
        row += 32 * s
        if row == 128:
            tile += 1
            row = 0
    T = tile + (1 if row > 0 else 0)

    # PSUM-bank assignment keyed by starting row-group (see module docstring):
    # four bank streams, each filling (q, w) slots; a full bank is copied out
    # and the stream opens a fresh one.
    pos_bank = np.zeros(NPOS, dtype=np.int64)
    pos_q = np.zeros(NPOS, dtype=np.int64)
    pos_w = np.zeros(NPOS, dtype=np.int64)
    stream_bank = [-1, -1, -1, -1]
    stream_cnt = [0, 0, 0, 0]
    next_bank = 0
    for j in range(NPOS):
        g = int(r_j[j]) // 32
        if stream_bank[g] < 0:
            stream_bank[g] = next_bank
            next_bank += 1
            stream_cnt[g] = 0
        cnt = stream_cnt[g]
        pos_bank[j] = stream_bank[g]
        pos_q[j] = cnt // 32
        pos_w[j] = cnt % 32
        stream_cnt[g] = cnt + 1
        if stream_cnt[g] == 128:
            stream_bank[g] = -1
    NB = next_bank
    return segs_per_core, NPOS, K, t_j, r_j, T, pos_bank, pos_q, pos_w, NB


def _pack_core(segs, NPOS, K, t_j, r_j, T, wbd, edge_features, edge_sources,
               hidden):
    # combined row data: 48 cols per row = 32 edge features + 16 source hidden
    D = np.zeros((T * 128, EFD + HID), dtype=np.float32)
    for j in range(min(len(segs), NPOS)):
        _, eids = segs[j]
        base = t_j[j] * 128 + r_j[j]
        D[base:base + len(eids), :EFD] = edge_features[eids]
        D[base:base + len(eids), EFD:] = hidden[edge_sources[eids]]
    # DRAM layout: [128 partitions, T*48 + 4096] so each partition is one
    # contiguous DMA span; SBUF tile t sits at free offset t*48; the
    # block-diag W rides in the tail so it shares the last chunk's DMA.
    d = D.reshape(T, 128, EFD + HID).swapaxes(0, 1).reshape(128, T * (EFD + HID))
    return np.ascontiguousarray(np.concatenate([d.astype(BF16), wbd], axis=1))


def _build_wbd(W):
    # Wbd[p=2h+half] [(q,f)=128, (q',mh)=128] = delta_qq' W[f, (mh+32*half)*16+h]
    wbd = np.zeros((32, 128, 128), dtype=np.float32)
    Wr = W.reshape(EFD, MSG, HID)                      # [f, m, h]
    for h in range(HID):
        for half in range(2):
            p = 2 * h + half
            blk = Wr[:, 32 * half:32 * half + 32, h]   # [f=32, mh=32]
            for q in range(4):
                wbd[p, 32 * q:32 * q + 32, 32 * q:32 * q + 32] = blk
    # DRAM layout [128, 32*128]: phase p at free offset 128p
    return np.ascontiguousarray(
        wbd.transpose(1, 0, 2).reshape(128, 32 * 128)).astype(BF16)


def _chunk_bounds(T):
    return [round(f * T) for f in CHUNK_FRACS]


def _build_program(NPOS, K, t_j, r_j, T, pos_bank, pos_q, pos_w, NB):
    import concourse.tile as tile
    from concourse import bacc, mybir

    RW = EFD + HID                   # 48 row cols (ef | nh)
    f32 = mybir.dt.float32
    bf16 = mybir.dt.bfloat16
    bounds = _chunk_bounds(T)

    nc = bacc.Bacc("TRN2", target_bir_lowering=False, debug=False,
                   num_devices=NCORES)
    data_dram = nc.dram_tensor("data", [128, T * RW + 32 * 128], bf16,
                               kind="ExternalInput").ap()
    out_dram = nc.dram_tensor("out", [128, 2 * NB * 32], f32,
                              kind="ExternalOutput").ap()

    with tile.TileContext(nc) as tc, ExitStack() as ctx:
        big = ctx.enter_context(tc.tile_pool(name="big", bufs=1))
        cpool = ctx.enter_context(tc.tile_pool(name="cps", bufs=CPBUFS,
                                               space="PSUM"))
        opool = ctx.enter_context(tc.tile_pool(name="ops", bufs=1, space="PSUM"))

        ch_sb = []
        for k in range(NCHUNK):
            lo, hi = bounds[k] * RW, bounds[k + 1] * RW
            if k == NCHUNK - 1:
                hi += 32 * 128       # wbd tail rides with the last chunk
            t = big.tile([128, hi - lo], bf16, tag=f"ch{k}", name=f"ch{k}")
            nc.sync.dma_start(t[:], data_dram[:, lo:hi])
            ch_sb.append(t)
        wbd_sb = ch_sb[-1][:, (bounds[NCHUNK] - bounds[NCHUNK - 1]) * RW:]

        # h-major: c_all[p, h, 32*b + w] so the W-stage moving is contiguous
        c_all = big.tile([128, HID, NB * 32], bf16, tag="call")

        # final bank of each row-group stream may be partially filled: zero
        # its c_all region so the prefix-copy below leaves no stale data
        gcount = {}
        for j in range(NPOS):
            b = int(pos_bank[j])
            gcount[b] = gcount.get(b, 0) + 1
        for b, cnt in gcount.items():
            if cnt < 128:
                nc.vector.memset(c_all[:, :, 32 * b:32 * (b + 1)], 0.0)

        chunk_of = np.searchsorted(np.array(bounds[1:]), t_j, side="right")
        stream_tile = [None, None, None, None]
        stream_n = [0, 0, 0, 0]

        def flush(g, b):
            # cps tile is [128, 32(w), 16(h)]; emit h-major into c_all
            src = stream_tile[g][:, :, :].transpose([0, 2, 1])
            nc.vector.tensor_copy(c_all[:, :, 32 * b:32 * b + 32], src)

        for j in range(NPOS):
            g = int(r_j[j]) // 32
            if stream_tile[g] is None:
                stream_tile[g] = cpool.tile([128, 32, HID], f32, tag="cps",
                                            name=f"cps_b{int(pos_bank[j])}")
                stream_n[g] = 0
            t, r, kk = int(t_j[j]), int(r_j[j]), int(K[j])
            ch = int(chunk_of[j])
            base = (t - bounds[ch]) * RW
            q, w = int(pos_q[j]), int(pos_w[j])
            lhsT = ch_sb[ch][r:r + kk, base:base + EFD]
            rhs = ch_sb[ch][r:r + kk, base + EFD:base + RW]
            out = stream_tile[g][32 * q:32 * q + 32, w, :]
            nc.tensor.matmul(out, lhsT, rhs, start=True, stop=True,
                             tile_position=(r, 32 * q))
            stream_n[g] += 1
            if stream_n[g] == 128:
                flush(g, int(pos_bank[j]))
                stream_tile[g] = None
        for g in range(4):           # flush partial final banks
            if stream_tile[g] is not None:
                b = [int(pos_bank[j]) for j in range(NPOS)
                     if int(r_j[j]) // 32 == g][-1]
                flush(g, b)

        out_sb = big.tile([128, 2 * NB * 32], f32, tag="outsb")
        for half in range(2):
            po = opool.tile([128, NB * 32], f32, tag=f"po{half}",
                            name=f"po{half}")
            for h in range(HID):
                p = 2 * h + half
                nc.tensor.matmul(
                    po[:], wbd_sb[:, 128 * p:128 * p + 128], c_all[:, h, :],
                    start=(h == 0), stop=(h == HID - 1))
            nc.vector.tensor_copy(
                out_sb[:, NB * 32 * half:NB * 32 * (half + 1)], po[:])
        nc.sync.dma_start(out_dram[:], out_sb[:])
    nc.compile()
    return nc


def _assemble(outs, segs_per_core, NPOS, pos_bank, pos_q, pos_w, NB):
    WND = NB * 32
    out = np.zeros((N_NODES, MSG), dtype=np.float32)
    for c in range(NCORES):
        out_sb = outs[c]
        pos_rows = np.empty((NPOS, MSG), dtype=np.float32)
        for half in range(2):
            pos_rows[:, 32 * half:32 * half + 32] = \
                out_sb[32 * pos_q[:, None] + np.arange(32)[None, :],
                       (WND * half + 32 * pos_bank + pos_w)[:, None]]
        segs = segs_per_core[c]
        for j in range(min(len(segs), NPOS)):
            n, _ = segs[j]
            out[n] += pos_rows[j]
    return out


def kernel(node_features, edge_features, edge_sources, edge_targets,
           hidden, initial, W, b):
    from concourse.bass_utils import run_bass_kernel_spmd

    edge_targets = np.asarray(edge_targets)
    edge_sources = np.asarray(edge_sources)
    edge_features = np.asarray(edge_features, dtype=np.float32)
    hidden = np.asarray(hidden, dtype=np.float32)
    W = np.asarray(W, dtype=np.float32)
    b = np.asarray(b, dtype=np.float32)

    key = edge_targets.tobytes()
    if key in _CACHE:
        layout, nc = _CACHE[key]
    else:
        layout = _build_layout(edge_targets)
        segs_per_core, NPOS, K, t_j, r_j, T, pos_bank, pos_q, pos_w, NB = layout
        assert K.max() <= 128
        nc = _build_program(NPOS, K, t_j, r_j, T, pos_bank, pos_q, pos_w, NB)
        _CACHE[key] = (layout, nc)
    segs_per_core, NPOS, K, t_j, r_j, T, pos_bank, pos_q, pos_w, NB = layout

    wbd = _build_wbd(W)
    in_maps = []
    for c in range(NCORES):
        data = _pack_core(segs_per_core[c], NPOS, K, t_j, r_j, T, wbd,
                          edge_features, edge_sources, hidden)
        in_maps.append({"data": data})

    res = run_bass_kernel_spmd(nc, in_maps, list(range(NCORES)))
    outs = [res.results[c]["out"] for c in range(NCORES)]
    out = _assemble(outs, segs_per_core, NPOS, pos_bank, pos_q, pos_w, NB)

    if np.any(b):
        # bias term: out[n] += (sum_{e->n} hidden[src e]) @ Br,
        # Br[h, m] = b[m*16+h].  (b is all-zero for this problem.)
        Br = b.reshape(MSG, HID).T.astype(np.float32)
        acc = np.zeros((N_NODES, HID), dtype=np.float32)
        np.add.at(acc, edge_targets, hidden[edge_sources])
        out += acc @ Br
    return out


# revision 15
# speedup vs baseline: 1.3585x; 1.3585x over previous
"""Trainium2 Bass kernel for nn_MessageLayer (GNN message passing), 8 NeuronCores.

Reference computation:
    edge_mat = (edge_features @ W + b).reshape(E, 64, 16)
    messages = einsum('emh,eh->em', edge_mat, hidden[edge_sources])
    out      = segment_sum(messages, edge_targets, num_segments=10000)

Algebraic restructure (cuts FLOPs 32x): since aggregation is linear,
    out[n, m] = sum_{f,h} W[f, m*16+h] * C[n, f, h],
    C[n, f, h] = sum_{e: tgt(e)=n} ef[e, f] * hidden[src(e), h]
C is built with one tiny PE matmul per target node (lhsT = that node's edge
features [deg, 32], rhs = gathered source hidden [deg, 16]) — segment
boundaries are known when the kernel is built, so they are baked into the
unrolled instruction stream.  Then out = C @ Wr as 32 accumulating matmuls
against a block-diagonal-packed W.

All matmul operands are bf16 (PSUM accumulation stays fp32): the PE streams
bf16 at 1 cycle/row vs fp32's 4, and DMA bytes halve.  Quantization error
~0.3% is far inside the 2e-2 gate.

Sharding: node-ownership (scatter-reduce by target) — each core receives
exactly the edges targeting its nodes, so no collective is needed.  Nodes
are dealt to cores snake-wise in descending-degree order, so the sorted
per-core segment-length profiles match almost exactly across cores and the
SPMD max-padding (K_j = max over cores of the j-th longest segment) is
minimal.

c_all is stored h-major ([128, 16, NB*32]) so the W-stage moving operand
c_all[:, h, :] is contiguous — a strided (h::16) moving was measured at
~2.7 ns/column vs ~0.8 contiguous.

Hardware constraint discovered empirically: matmuls whose stationary tiles
sit on disjoint row-groups execute CONCURRENTLY, and two concurrent matmuls
draining into the same PSUM bank wedge the device (NRT_EXEC_UNIT_
UNRECOVERABLE).  C-bank assignment is therefore keyed by starting row-group:
matmuls sharing a row-group serialize on the PE and may share a bank;
different row-groups always land in different banks.
"""
import numpy as np
import ml_dtypes
from contextlib import ExitStack

BF16 = ml_dtypes.bfloat16

N_NODES = 10000
N_EDGES = 320000
HID = 16
MSG = 64
EFD = 32
NCORES = 8
# progressive input chunks: small first chunks so the PE starts early and
# never outruns the DMA at a chunk boundary
CHUNK_FRACS = (0.0, 0.02, 0.05, 0.09, 0.14, 0.20, 0.27, 0.35, 0.43, 0.52,
               0.61, 0.70, 0.80, 0.90, 1.0)
NCHUNK = len(CHUNK_FRACS) - 1
CPBUFS = 6                       # PSUM tiles for C banks (+2 for the W stage)

_CACHE = {}


def _build_layout(edge_targets):
    """Per-core segment lists + the shared (SPMD-uniform) layout."""
    deg = np.bincount(edge_targets, minlength=N_NODES)
    order = np.argsort(-deg, kind="stable")      # nodes by degree desc
    node_core = np.empty(N_NODES, dtype=np.int64)
    snake = list(range(NCORES)) + list(range(NCORES - 1, -1, -1))
    for i, n in enumerate(order):
        node_core[n] = snake[i % (2 * NCORES)]

    # bucket edge ids by target, in one pass
    order_e = np.argsort(edge_targets, kind="stable")
    tgt_sorted = edge_targets[order_e]
    uniq, starts = np.unique(tgt_sorted, return_index=True)
    bounds = list(starts) + [len(tgt_sorted)]

    # One matmul call per segment; PE issue rate (~35 ns/call, independent of
    # segment length) dominates, so do NOT split finer than necessary.
    segs_per_core = [[] for _ in range(NCORES)]
    for i, n in enumerate(uniq):
        s, e = bounds[i], bounds[i + 1]
        c = node_core[n]
        while e - s > 128:          # split over-long segments; host re-adds
            segs_per_core[c].append((int(n), order_e[s:s + 128]))
            s += 128
        segs_per_core[c].append((int(n), order_e[s:e]))
    for c in range(NCORES):
        segs_per_core[c].sort(key=lambda t: -len(t[1]))

    nseg = max(len(s) for s in segs_per_core)
    NPOS = ((nseg + 127) // 128) * 128
    K = np.ones(NPOS, dtype=np.int64)
    for segs in segs_per_core:
        for j, (_, e) in enumerate(segs):
            K[j] = max(K[j], len(e))

    # pack positions into 128-row tiles at 32-aligned row bases
    t_j = np.zeros(NPOS, dtype=np.int64)
    r_j = np.zeros(NPOS, dtype=np.int64)
    tile, row = 0, 0
    for j in range(NPOS):
        s = (int(K[j]) + 31) // 32
        if row + 32 * s > 128:
            tile += 1
            row = 0
        t_j[j], r_j[j] = tile, row
        row += 32 * s
        if row == 128:
            tile += 1
            row = 0
    T = tile + (1 if row > 0 else 0)

    # PSUM-bank assignment keyed by starting row-group (see module docstring):
    # four bank streams, each filling (q, w) slots; a full bank is copied out
    # and the stream opens a fresh one.
    pos_bank = np.zeros(NPOS, dtype=np.int64)
    pos_q = np.zeros(NPOS, dtype=np.int64)
    pos_w = np.zeros(NPOS, dtype=np.int64)
    stream_bank = [-1, -1, -1, -1]
    stream_cnt = [0, 0, 0, 0]
    next_bank = 0
    for j in range(NPOS):
        g = int(r_j[j]) // 32
        if stream_bank[g] < 0:
            stream_bank[g] = next_bank
            next_bank += 1
            stream_cnt[g] = 0
        cnt = stream_cnt[g]
        pos_bank[j] = stream_bank[g]
        pos_q[j] = cnt // 32
        pos_w[j] = cnt % 32
        stream_cnt[g] = cnt + 1
        if stream_cnt[g] == 128:
            stream_bank[g] = -1
    NB = next_bank
    return segs_per_core, NPOS, K, t_j, r_j, T, pos_bank, pos_q, pos_w, NB


def _pack_core(segs, NPOS, K, t_j, r_j, T, wbd, edge_features, edge_sources,
               hidden):
    # combined row data: 48 cols per row = 32 edge features + 16 source hidden
    D = np.zeros((T * 128, EFD + HID), dtype=np.float32)
    for j in range(min(len(segs), NPOS)):
        _, eids = segs[j]
        base = t_j[j] * 128 + r_j[j]
        D[base:base + len(eids), :EFD] = edge_features[eids]
        D[base:base + len(eids), EFD:] = hidden[edge_sources[eids]]
    # DRAM layout: [128 partitions, T*48 + 4096] so each partition is one
    # contiguous DMA span; SBUF tile t sits at free offset t*48; the
    # block-diag W rides in the tail so it shares the last chunk's DMA.
    d = D.reshape(T, 128, EFD + HID).swapaxes(0, 1).reshape(128, T * (EFD + HID))
    return np.ascontiguousarray(np.concatenate([d.astype(BF16), wbd], axis=1))


def _build_wbd(W):
    # Wbd[p=2h+half] [(q,f)=128, (q',mh)=128] = delta_qq' W[f, (mh+32*half)*16+h]
    wbd = np.zeros((32, 128, 128), dtype=np.float32)
    Wr = W.reshape(EFD, MSG, HID)                      # [f, m, h]
    for h in range(HID):
        for half in range(2):
            p = 2 * h + half
            blk = Wr[:, 32 * half:32 * half + 32, h]   # [f=32, mh=32]
            for q in range(4):
                wbd[p, 32 * q:32 * q + 32, 32 * q:32 * q + 32] = blk
    # DRAM layout [128, 32*128]: phase p at free offset 128p
    return np.ascontiguousarray(
        wbd.transpose(1, 0, 2).reshape(128, 32 * 128)).astype(BF16)


def _chunk_bounds(T):
    return [round(f * T) for f in CHUNK_FRACS]


def _build_program(NPOS, K, t_j, r_j, T, pos_bank, pos_q, pos_w, NB):
    import concourse.tile as tile
    from concourse import bacc, mybir

    RW = EFD + HID                   # 48 row cols (ef | nh)
    f32 = mybir.dt.float32
    bf16 = mybir.dt.bfloat16
    bounds = _chunk_bounds(T)

    nc = bacc.Bacc("TRN2", target_bir_lowering=False, debug=False,
                   num_devices=NCORES)
    data_dram = nc.dram_tensor("data", [128, T * RW + 32 * 128], bf16,
                               kind="ExternalInput").ap()
    out_dram = nc.dram_tensor("out", [128, 2 * NB * 32], f32,
                              kind="ExternalOutput").ap()

    with tile.TileContext(nc) as tc, ExitStack() as ctx:
        big = ctx.enter_context(tc.tile_pool(name="big", bufs=1))
        cpool = ctx.enter_context(tc.tile_pool(name="cps", bufs=CPBUFS,
                                               space="PSUM"))
        opool = ctx.enter_context(tc.tile_pool(name="ops", bufs=1, space="PSUM"))

        ch_sb = []
        for k in range(NCHUNK):
            lo, hi = bounds[k] * RW, bounds[k + 1] * RW
            if k == NCHUNK - 1:
                hi += 32 * 128       # wbd tail rides with the last chunk
            t = big.tile([128, hi - lo], bf16, tag=f"ch{k}", name=f"ch{k}")
            # alternate the two HW DGE queues (SP / Activation)
            eng = nc.sync if k % 2 == 0 else nc.scalar
            eng.dma_start(t[:], data_dram[:, lo:hi])
            ch_sb.append(t)
        wbd_sb = ch_sb[-1][:, (bounds[NCHUNK] - bounds[NCHUNK - 1]) * RW:]

        # h-major: c_all[p, h, 32*b + w] so the W-stage moving is contiguous
        c_all = big.tile([128, HID, NB * 32], bf16, tag="call")

        # final bank of each row-group stream may be partially filled: zero
        # its c_all region so the prefix-copy below leaves no stale data
        gcount = {}
        for j in range(NPOS):
            b = int(pos_bank[j])
            gcount[b] = gcount.get(b, 0) + 1
        for b, cnt in gcount.items():
            if cnt < 128:
                nc.vector.memset(c_all[:, :, 32 * b:32 * (b + 1)], 0.0)

        chunk_of = np.searchsorted(np.array(bounds[1:]), t_j, side="right")
        stream_tile = [None, None, None, None]
        stream_n = [0, 0, 0, 0]

        def flush(g, b):
            # cps tile is [128, 32(w), 16(h)]; emit h-major into c_all.
            # Alternate DVE/Activation so the final flushes don't serialize
            # on one engine right before the W stage.
            src = stream_tile[g][:, :, :].transpose([0, 2, 1])
            if b % 2 == 0:
                nc.vector.tensor_copy(c_all[:, :, 32 * b:32 * b + 32], src)
            else:
                nc.scalar.copy(c_all[:, :, 32 * b:32 * b + 32], src)

        for j in range(NPOS):
            g = int(r_j[j]) // 32
            if stream_tile[g] is None:
                stream_tile[g] = cpool.tile([128, 32, HID], f32, tag="cps",
                                            name=f"cps_b{int(pos_bank[j])}")
                stream_n[g] = 0
            t, r, kk = int(t_j[j]), int(r_j[j]), int(K[j])
            ch = int(chunk_of[j])
            base = (t - bounds[ch]) * RW
            q, w = int(pos_q[j]), int(pos_w[j])
            lhsT = ch_sb[ch][r:r + kk, base:base + EFD]
            rhs = ch_sb[ch][r:r + kk, base + EFD:base + RW]
            out = stream_tile[g][32 * q:32 * q + 32, w, :]
            nc.tensor.matmul(out, lhsT, rhs, start=True, stop=True,
                             tile_position=(r, 32 * q))
            stream_n[g] += 1
            if stream_n[g] == 128:
                flush(g, int(pos_bank[j]))
                stream_tile[g] = None
        for g in range(4):           # flush partial final banks
            if stream_tile[g] is not None:
                b = [int(pos_bank[j]) for j in range(NPOS)
                     if int(r_j[j]) // 32 == g][-1]
                flush(g, b)

        out_sb = big.tile([128, 2 * NB * 32], f32, tag="outsb")
        for half in range(2):
            po = opool.tile([128, NB * 32], f32, tag=f"po{half}",
                            name=f"po{half}")
            for h in range(HID):
                p = 2 * h + half
                nc.tensor.matmul(
                    po[:], wbd_sb[:, 128 * p:128 * p + 128], c_all[:, h, :],
                    start=(h == 0), stop=(h == HID - 1))
            sl = slice(NB * 32 * half, NB * 32 * (half + 1))
            if half == 0:
                nc.vector.tensor_copy(out_sb[:, sl], po[:])
            else:
                nc.scalar.copy(out_sb[:, sl], po[:])
            # half-0 writeback DMA overlaps half-1 W matmuls
            (nc.sync if half == 0 else nc.scalar).dma_start(
                out_dram[:, sl], out_sb[:, sl])
    nc.compile()
    return nc


def _assemble(outs, segs_per_core, NPOS, pos_bank, pos_q, pos_w, NB):
    WND = NB * 32
    out = np.zeros((N_NODES, MSG), dtype=np.float32)
    for c in range(NCORES):
        out_sb = outs[c]
        pos_rows = np.empty((NPOS, MSG), dtype=np.float32)
        for half in range(2):
            pos_rows[:, 32 * half:32 * half + 32] = \
                out_sb[32 * pos_q[:, None] + np.arange(32)[None, :],
                       (WND * half + 32 * pos_bank + pos_w)[:, None]]
        segs = segs_per_core[c]
        for j in range(min(len(segs), NPOS)):
            n, _ = segs[j]
            out[n] += pos_rows[j]
    return out


def kernel(node_features, edge_features, edge_sources, edge_targets,
           hidden, initial, W, b):
    from concourse.bass_utils import run_bass_kernel_spmd

    edge_targets = np.asarray(edge_targets)
    edge_sources = np.asarray(edge_sources)
    edge_features = np.asarray(edge_features, dtype=np.float32)
    hidden = np.asarray(hidden, dtype=np.float32)
    W = np.asarray(W, dtype=np.float32)
    b = np.asarray(b, dtype=np.float32)

    key = edge_targets.tobytes()
    if key in _CACHE:
        layout, nc = _CACHE[key]
    else:
        layout = _build_layout(edge_targets)
        segs_per_core, NPOS, K, t_j, r_j, T, pos_bank, pos_q, pos_w, NB = layout
        assert K.max() <= 128
        nc = _build_program(NPOS, K, t_j, r_j, T, pos_bank, pos_q, pos_w, NB)
        _CACHE[key] = (layout, nc)
    segs_per_core, NPOS, K, t_j, r_j, T, pos_bank, pos_q, pos_w, NB = layout

    wbd = _build_wbd(W)
    in_maps = []
    for c in range(NCORES):
        data = _pack_core(segs_per_core[c], NPOS, K, t_j, r_j, T, wbd,
                          edge_features, edge_sources, hidden)
        in_maps.append({"data": data})

    res = run_bass_kernel_spmd(nc, in_maps, list(range(NCORES)))
    outs = [res.results[c]["out"] for c in range(NCORES)]
    out = _assemble(outs, segs_per_core, NPOS, pos_bank, pos_q, pos_w, NB)

    if np.any(b):
        # bias term: out[n] += (sum_{e->n} hidden[src e]) @ Br,
        # Br[h, m] = b[m*16+h].  (b is all-zero for this problem.)
        Br = b.reshape(MSG, HID).T.astype(np.float32)
        acc = np.zeros((N_NODES, HID), dtype=np.float32)
        np.add.at(acc, edge_targets, hidden[edge_sources])
        out += acc @ Br
    return out
